# revision 1
# baseline (speedup 1.0000x reference)
"""Trainium2 Bass kernel for nn_MetaSelectTarget (FPN level assignment).

Strategy (final):
  - Data-parallel over batch: B=8 images -> 8 NeuronCores, one image each.
  - No dense pass: everything is computed on gathered windows (a gt box's
    shrunk projection spans at most w in {9,5,3,2,2} cells per side).
  - 128 partitions: box g lives on partitions g and g+64, each half handling
    half of the window rows.  A unified column layout packs all levels'
    window cells into 74 columns, so window math runs as batched ops on
    [128, 74/148/296] tiles with per-column constants from inline tables.
  - HW indirect DMA reads ONE contiguous block per partition from idx[p,0],
    so: 12 class row-block gathers, 5 label-shifted row-block gathers for
    level-0 psel (psel then falls out of a strided view), 5 regr band
    gathers + one static gpsimd indirect_copy to extract channels, and one
    more indirect_copy to broadcast per-level scalars to the ragged layout.
  - psel for levels 1-4 comes from a one-hot multiply + per-cell class
    reduce on DVE; the f0 term sum_c p^2 ln(1-p) is a per-cell class reduce.
  - argmin over levels + padding-box masking on-device; host concatenates
    the 8 per-core [64] outputs.
"""

import numpy as np

import concourse.bass as bass
import concourse.bacc as bacc
import concourse.tile as tile
from concourse import mybir
from contextlib import ExitStack

f32 = mybir.dt.float32
i32 = mybir.dt.int32
u16 = mybir.dt.uint16
AF = mybir.ActivationFunctionType
OP = mybir.AluOpType
AX = mybir.AxisListType

# Problem constants (hardcoded per spec)
G = 64
C = 80
FS = [(128, 128), (64, 64), (32, 32), (16, 16), (8, 8)]
STRIDES = [8.0, 16.0, 32.0, 64.0, 128.0]
ROWOFS = [0, 16384, 20480, 21504, 21760]
NLOC = 21824
W = [9, 5, 3, 2, 2]       # true max window span per level
NH = [5, 3, 2, 1, 1]      # window row-blocks handled per partition half
WSLOT = [9, 5, 3, 2, 2]   # column slots per row-block (== W)
EPS = 1e-7
BIG = 1e7
BIG2 = 16.0               # argmin tie-break penalty (> NLEV)
NLEV = 5

# unified column layout: one column per (level, row-block, slot)
_cols = []      # (level, block, slot)
for _l in range(NLEV):
    for _b in range(NH[_l]):
        for _j in range(WSLOT[_l]):
            _cols.append((_l, _b, _j))
NW = len(_cols)           # 70
assert NW == 70
_lvl_cols = [sum(NH[l] * WSLOT[l] for l in range(lv)) for lv in range(NLEV + 1)]
# (level, block) -> column index of slot 0, in layout order
_J0 = [(l, b, k) for k, (l, b, j) in enumerate(_cols) if j == 0]
assert len(_J0) == 12
_J0_FIRST = {l: s for s, (ll, b, k) in reversed(list(enumerate(_J0))) for l in [ll] if b == 0}
# regr band layout: per level ((nh-1)*fw + w) cells * 4 channels
_blen = [((NH[l] - 1) * FS[l][1] + W[l]) * 4 for l in range(NLEV)]
_bofs = [sum(_blen[:l]) for l in range(NLEV + 1)]
NBAND = _bofs[NLEV]       # 2772
# nominal extents of the [nh, fw, 4] views need a tail pad
NBAND_ALLOC = max(_bofs[l] + NH[l] * FS[l][1] * 4 for l in range(NLEV))


def _wrap_idx(mapv):
    """Per-16-partition wrapped uint16 index tensor for gpsimd.indirect_copy:
    unwrapped[i] lives at (i % 16, i // 16)."""
    n = len(mapv)
    s = (n + 15) // 16
    a = np.zeros((128, s), np.uint16)
    for i, m in enumerate(mapv):
        a[np.arange(8) * 16 + i % 16, i // 16] = m
    return a


def build_nc(num_devices=8):
    nc = bacc.Bacc("TRN2", target_bir_lowering=False, num_devices=num_devices)

    cls_b = nc.dram_tensor("cls_b", [NLOC, C], f32, kind="ExternalInput")
    regr_b = nc.dram_tensor("regr_b", [NLOC, 4], f32, kind="ExternalInput")
    gt_b = nc.dram_tensor("gt_b", [G, 5], f32, kind="ExternalInput")
    out_lvl = nc.dram_tensor("out_lvl", [G], i32, kind="ExternalOutput")

    cls_flat = cls_b.ap().rearrange("n c -> (n c)")[None, :]
    regr_flat = regr_b.ap().rearrange("n c -> (n c)")[None, :]

    # ---- inline constants --------------------------------------------------
    # V-chain tables [128, NLEV*4]; cols j: 0=x1, 1=y1, 2=x2, 3=y2
    recip = np.zeros((128, NLEV, 4), np.float32)
    maskF = np.zeros((128, NLEV, 4), np.float32)
    maskC = np.zeros((128, NLEV, 4), np.float32)
    clo = np.zeros((128, NLEV, 4), np.float32)
    chi = np.zeros((128, NLEV, 4), np.float32)
    shi = np.zeros((128, NLEV, 4), np.float32)
    for l in range(NLEV):
        fh, fw = FS[l]
        w = W[l]
        recip[:, l, :] = 1.0 / STRIDES[l]
        maskF[:, l, 0] = maskF[:, l, 1] = 1.0
        maskC[:, l, 2] = maskC[:, l, 3] = 1.0
        clo[:, l, :] = [0.0, 0.0, 1.0, 1.0]
        chi[:, l, :] = [fw - 1, fh - 1, fw, fh]
        shi[:, l, :] = [fw - w, fh - w, 1e9, 1e9]

    # per-column tables
    r74 = np.zeros((128, NW), np.float32)     # global window row index
    j74 = np.zeros((128, NW), np.float32)     # slot (x offset in window)
    inv4s296 = np.zeros((128, 4 * NW), np.float32)
    for k, (l, b, j) in enumerate(_cols):
        r74[0:64, k] = b
        r74[64:128, k] = NH[l] + b
        j74[:, k] = j
        for ch in range(4):
            inv4s296[:, ch * NW + k] = 1.0 / (4.0 * STRIDES[l])

    # row-block tables [128, 12]
    r12 = np.zeros((128, 12), np.float32)
    fw12 = np.zeros((128, 12), np.float32)
    fhm112 = np.zeros((128, 12), np.float32)
    rofs12 = np.zeros((128, 12), np.float32)
    for s, (l, b, k) in enumerate(_J0):
        r12[0:64, s] = b
        r12[64:128, s] = NH[l] + b
        fw12[:, s] = FS[l][1]
        fhm112[:, s] = FS[l][0] - 1
        rofs12[:, s] = ROWOFS[l]

    scale24 = np.tile(np.array([float(C)] * 12 + [4.0] * 12, np.float32), (128, 1))
    cconst = np.tile(np.arange(C, dtype=np.float32), (128, 1))
    clb5 = np.tile(np.arange(NLEV, dtype=np.float32) + BIG2, (128, 1))
    constm1 = np.full((128, 1), -1.0, np.float32)
    consteps = np.full((128, 1), EPS, np.float32)

    consts = np.concatenate(
        [recip.reshape(128, -1), maskF.reshape(128, -1), maskC.reshape(128, -1),
         clo.reshape(128, -1), chi.reshape(128, -1), shi.reshape(128, -1),
         r74, j74, inv4s296,
         r12, fw12, fhm112, rofs12,
         scale24, cconst, clb5, constm1, consteps], axis=1)
    t_consts = nc.inline_tensor(consts, "c_all")
    NCONST = consts.shape[1]

    # XSYS: out blocks [XS74, YS74, X1V74, Y1V74, X2V74, Y2V74, XS12, YS12]
    # from SVVR [128, 40] (SV at 0:20, VR at 20:40; (l j) layout)
    xsys_map = []
    for q in range(6):
        for k, (l, b, j) in enumerate(_cols):
            xsys_map.append([4 * l + 0, 4 * l + 1, 20 + 4 * l + 0, 20 + 4 * l + 1,
                             20 + 4 * l + 2, 20 + 4 * l + 3][q])
    for s, (l, b, k) in enumerate(_J0):
        xsys_map.append(4 * l + 0)
    for s, (l, b, k) in enumerate(_J0):
        xsys_map.append(4 * l + 1)
    # small map (XS12/YS12) gates the index chain; the 6x70 blocks don't
    t_xsys_idx_s = nc.inline_tensor(_wrap_idx(xsys_map[6 * NW:]), "xsys_idx_s")
    t_xsys_idx_b = nc.inline_tensor(_wrap_idx(xsys_map[:6 * NW]), "xsys_idx_b")
    NXS = len(xsys_map)  # 444

    NP4 = 4 * NW

    NPW = _lvl_cols[NLEV] * C   # 5600

    with tile.TileContext(nc) as tc, ExitStack() as ctx:
        pc = ctx.enter_context(tc.tile_pool(name="pc", bufs=1))

        GT = pc.tile([128, 5], f32)
        nc.sync.dma_start(GT[0:64, :], gt_b[:])
        nc.sync.dma_start(GT[64:128, :], gt_b[:])
        CST = pc.tile([128, NCONST], f32)
        nc.sync.dma_start(CST[:, 0:120], t_consts[:, 0:120])
        nc.sync.dma_start(CST[:, 120:NCONST], t_consts[:, 120:NCONST])
        # dummy ops: force the Ln/Square ACT table load at t~0, not lazily
        # inside the first data-dependent activation
        WARM = pc.tile([1, 2], f32)
        nc.vector.memset(WARM[:], 0.5)
        nc.scalar.activation(WARM[:, 0:1], WARM[:, 0:1], AF.Ln)
        nc.scalar.activation(WARM[:, 1:2], WARM[:, 1:2], AF.Square)
        off = 0
        def _cview(n):
            nonlocal off
            v = CST[:, off:off + n]
            off += n
            return v
        RECIP = _cview(NLEV * 4)
        MASKF = _cview(NLEV * 4)
        MASKC = _cview(NLEV * 4)
        CLO = _cview(NLEV * 4)
        CHI = _cview(NLEV * 4)
        SHI = _cview(NLEV * 4)
        R74 = _cview(NW)
        J74 = _cview(NW)
        INV4S296 = _cview(4 * NW)
        R12 = _cview(12)
        FW12 = _cview(12)
        FHM112 = _cview(12)
        ROFS12 = _cview(12)
        SCALE24 = _cview(24)
        CCONST = _cview(C)
        CLB5 = _cview(NLEV)
        CONSTM1 = _cview(1)
        CONSTEPS = _cview(1)

        XSYS_IDX_S = pc.tile([128, 2], u16)
        nc.sync.dma_start(XSYS_IDX_S[:], t_xsys_idx_s[:])
        XSYS_IDX_B = pc.tile([128, (6 * NW + 15) // 16], u16)
        nc.sync.dma_start(XSYS_IDX_B[:], t_xsys_idx_b[:])

        LBL = pc.tile([128, 1], f32)
        nc.vector.tensor_scalar(LBL[:], GT[:, 4:5], 0.0, float(C - 1), OP.max, OP.min)

        # ---- box math (both halves compute identical values) ---------------
        # shrunk box edges: q = 0.6*edge + 0.4*opposite
        GTSW = pc.tile([128, 4], f32)
        nc.vector.tensor_copy(GTSW[:, 0:2], GT[:, 2:4])
        nc.vector.tensor_copy(GTSW[:, 2:4], GT[:, 0:2])
        Q = pc.tile([128, 4], f32)
        nc.vector.tensor_scalar(Q[:], GTSW[:], 0.4, None, OP.mult)
        nc.vector.scalar_tensor_tensor(Q[:], GT[:, 0:4], 0.6, Q[:], OP.mult, OP.add)

        # V[p, l, j] = Q[p, j] / stride_l ; robust floor/ceil ; clip
        SVVR = pc.tile([128, 40], f32)
        SV = SVVR[:, 0:20]
        VR = SVVR[:, 20:40]
        V = pc.tile([128, NLEV * 4], f32)
        nc.vector.tensor_tensor(
            out=V[:].rearrange("g (l j) -> g l j", j=4),
            in0=Q[:, None, :].to_broadcast([128, NLEV, 4]),
            in1=RECIP.rearrange("g (l j) -> g l j", j=4),
            op=OP.mult,
        )
        VI = pc.tile([128, NLEV * 4], i32)
        nc.vector.tensor_copy(VI[:], V[:])
        VF = pc.tile([128, NLEV * 4], f32)
        nc.vector.tensor_copy(VF[:], VI[:])
        GG = pc.tile([128, NLEV * 4], f32)
        nc.vector.tensor_tensor(out=GG[:], in0=VF[:], in1=V[:], op=OP.is_gt)
        LL = pc.tile([128, NLEV * 4], f32)
        nc.vector.tensor_tensor(out=LL[:], in0=VF[:], in1=V[:], op=OP.is_lt)
        nc.vector.tensor_tensor(out=GG[:], in0=GG[:], in1=MASKF, op=OP.mult)
        nc.vector.tensor_tensor(out=LL[:], in0=LL[:], in1=MASKC, op=OP.mult)
        nc.vector.tensor_tensor(out=VR, in0=VF[:], in1=GG[:], op=OP.subtract)
        nc.vector.tensor_tensor(out=VR, in0=VR, in1=LL[:], op=OP.add)
        nc.vector.tensor_tensor(out=VR, in0=VR, in1=CLO, op=OP.max)
        nc.vector.tensor_tensor(out=VR, in0=VR, in1=CHI, op=OP.min)
        nc.vector.tensor_tensor(out=SV, in0=VR, in1=SHI, op=OP.min)

        # ---- broadcast per-level scalars to columns (one static gather) ----
        XSYS = pc.tile([128, NXS], f32)
        nc.gpsimd.indirect_copy(XSYS[:, 6 * NW:NXS], SVVR[:], XSYS_IDX_S[:], True)
        nc.gpsimd.indirect_copy(XSYS[:, 0:6 * NW], SVVR[:], XSYS_IDX_B[:], True)
        XS74 = XSYS[:, 0 * NW:1 * NW]
        YS74 = XSYS[:, 1 * NW:2 * NW]
        X1Y1V = XSYS[:, 2 * NW:4 * NW]
        X2Y2V = XSYS[:, 4 * NW:6 * NW]
        XS12 = XSYS[:, 6 * NW:6 * NW + 12]
        YS12 = XSYS[:, 6 * NW + 12:6 * NW + 24]

        # ---- row-block element indices (short critical path) ---------------
        # cell = min(ys + r, fh-1)*fw + xs + rowofs  (ys + r >= 0 always)
        CELL12 = pc.tile([128, 12], f32)
        nc.vector.tensor_tensor(out=CELL12[:], in0=YS12, in1=R12, op=OP.add)
        nc.vector.tensor_tensor(out=CELL12[:], in0=CELL12[:], in1=FHM112, op=OP.min)
        nc.vector.tensor_tensor(out=CELL12[:], in0=CELL12[:], in1=FW12, op=OP.mult)
        nc.vector.tensor_tensor(out=CELL12[:], in0=CELL12[:], in1=XS12, op=OP.add)
        nc.vector.tensor_tensor(out=CELL12[:], in0=CELL12[:], in1=ROFS12, op=OP.add)
        RIALLF = pc.tile([128, 24], f32)   # [cell*C | cell*4]
        nc.vector.tensor_tensor(
            out=RIALLF[:].rearrange("g (q s) -> g q s", s=12),
            in0=CELL12[:, None, :].to_broadcast([128, 2, 12]),
            in1=SCALE24.rearrange("g (q s) -> g q s", s=12), op=OP.mult)
        RIALL = pc.tile([128, 24], i32)
        nc.vector.tensor_copy(RIALL[:], RIALLF[:])
        RI80 = RIALL[:, 0:12]
        RIB12 = RIALL[:, 12:24]

        # label / psel-shifted level-0 indices
        PSIDXF = pc.tile([128, 5], f32)
        nc.vector.tensor_tensor(out=PSIDXF[:], in0=RIALLF[:, 0:5],
                                in1=LBL[:, 0:1].to_broadcast([128, 5]), op=OP.add)
        PSIDX = pc.tile([128, 5], i32)
        nc.vector.tensor_copy(PSIDX[:], PSIDXF[:])

        # ---- gathers (SWDGE / Pool): one contiguous block per partition ----
        PWALL = pc.tile([128, NPW], f32)
        RBAND = pc.tile([128, NBAND_ALLOC], f32)
        PSH = pc.tile([128, 5 * 720], f32)
        P4T = pc.tile([128, NP4], f32)
        for s, (l, b, k) in enumerate(_J0):
            if l > 0:
                continue
            ea = (_lvl_cols[l] + b * WSLOT[l]) * C
            nc.gpsimd.indirect_dma_start(
                out=PWALL[:, ea:ea + WSLOT[l] * C], out_offset=None, in_=cls_flat,
                in_offset=bass.IndirectOffsetOnAxis(ap=RI80[:, s:s + 1], axis=1))
        for s, (l, b, k) in enumerate(_J0):
            if l == 0:
                continue
            ea = (_lvl_cols[l] + b * WSLOT[l]) * C
            nc.gpsimd.indirect_dma_start(
                out=PWALL[:, ea:ea + WSLOT[l] * C], out_offset=None, in_=cls_flat,
                in_offset=bass.IndirectOffsetOnAxis(ap=RI80[:, s:s + 1], axis=1))
        for b in range(5):
            nc.gpsimd.indirect_dma_start(
                out=PSH[:, b * 720:(b + 1) * 720], out_offset=None, in_=cls_flat,
                in_offset=bass.IndirectOffsetOnAxis(ap=PSIDX[:, b:b + 1], axis=1))
        for l in range(NLEV):
            s = _J0_FIRST[l]
            nc.gpsimd.indirect_dma_start(
                out=RBAND[:, _bofs[l]:_bofs[l] + _blen[l]], out_offset=None,
                in_=regr_flat,
                in_offset=bass.IndirectOffsetOnAxis(ap=RIB12[:, s:s + 1], axis=1))

        # ---- window mask [128, NW] -----------------------------------------
        XJROWY = pc.tile([128, 2 * NW], f32)
        nc.vector.tensor_tensor(out=XJROWY[:, 0:NW], in0=XS74, in1=J74, op=OP.add)
        nc.vector.tensor_tensor(out=XJROWY[:, NW:2 * NW], in0=YS74, in1=R74, op=OP.add)
        XJ74 = XJROWY[:, 0:NW]
        ROWY = XJROWY[:, NW:2 * NW]
        MGE = pc.tile([128, 2 * NW], f32)
        nc.vector.tensor_tensor(out=MGE[:], in0=XJROWY[:], in1=X1Y1V, op=OP.is_ge)
        MLT = pc.tile([128, 2 * NW], f32)
        nc.vector.tensor_tensor(out=MLT[:], in0=XJROWY[:], in1=X2Y2V, op=OP.is_lt)
        nc.vector.tensor_tensor(out=MGE[:], in0=MGE[:], in1=MLT[:], op=OP.mult)
        M74 = pc.tile([128, NW], f32)
        nc.vector.tensor_tensor(out=M74[:], in0=MGE[:, 0:NW], in1=MGE[:, NW:2 * NW],
                                op=OP.mult)

        # empty / denom per level [128, NLEV] (rows 0:64 used at the end)
        VR3 = VR.rearrange("g (l j) -> g l j", j=4)
        x1v, y1v, x2v, y2v = VR3[:, :, 0], VR3[:, :, 1], VR3[:, :, 2], VR3[:, :, 3]
        EX = pc.tile([128, NLEV], f32)
        nc.vector.tensor_tensor(out=EX[:], in0=x1v, in1=x2v, op=OP.is_equal)
        EY = pc.tile([128, NLEV], f32)
        nc.vector.tensor_tensor(out=EY[:], in0=y1v, in1=y2v, op=OP.is_equal)
        EMX = pc.tile([128, NLEV], f32)
        nc.vector.tensor_tensor(out=EMX[:], in0=EX[:], in1=EY[:], op=OP.max)
        DX = pc.tile([128, NLEV], f32)
        nc.vector.tensor_tensor(out=DX[:], in0=x2v, in1=x1v, op=OP.subtract)
        DY = pc.tile([128, NLEV], f32)
        nc.vector.tensor_tensor(out=DY[:], in0=y2v, in1=y1v, op=OP.subtract)
        DN = pc.tile([128, NLEV], f32)
        nc.vector.tensor_tensor(out=DN[:], in0=DX[:], in1=DY[:], op=OP.mult)
        nc.vector.tensor_scalar(DN[:], DN[:], 1.0, None, OP.max)
        RECDN = pc.tile([128, NLEV], f32)
        nc.vector.reciprocal(RECDN[:], DN[:])

        ONEHOT = pc.tile([128, C], f32)
        nc.vector.tensor_tensor(out=ONEHOT[:], in0=CCONST,
                                in1=LBL[:, 0:1].to_broadcast([128, C]), op=OP.is_equal)
        SABS = pc.tile([128, 1], f32)
        nc.vector.tensor_reduce(SABS[:], GT[:, 0:4], axis=AX.X, op=OP.add,
                                apply_absolute_value=True)
        NV = pc.tile([128, 1], i32)
        nc.vector.tensor_scalar(NV[:], SABS[:], 0.0, None, OP.is_le)

        # ---- per-cell class reduces: f0 term and psel -----------------------
        # F0W = sum_c p^2 ln(1-p) ; PSEL[45:74] = sum_c p * onehot
        T1 = pc.tile([128, NPW], f32)
        T2 = pc.tile([128, NPW], f32)
        P3 = pc.tile([128, (_lvl_cols[NLEV] - _lvl_cols[1]) * C], f32)
        F0W = pc.tile([128, NW], f32)
        PSEL = pc.tile([128, NW], f32)
        # level 0 psel from the label-shifted gather (stride-C view)
        PSHv = PSH[:].rearrange("g (b j c) -> g b j c", j=9, c=C)
        nc.scalar.copy(
            PSEL[:, 0:45].rearrange("g (b j) -> g b j", j=9), PSHv[:, :, :, 0])
        regions = [(b * W[0] * C, (b + 1) * W[0] * C, b * W[0], (b + 1) * W[0])
                   for b in range(NH[0])]
        for l in range(1, NLEV):
            regions.append((_lvl_cols[l] * C, _lvl_cols[l + 1] * C,
                            _lvl_cols[l], _lvl_cols[l + 1]))
        for (ea, eb, ca, cb) in regions:
            if ca >= 45:
                nc.vector.tensor_tensor(
                    out=P3[:, ea - _lvl_cols[1] * C:eb - _lvl_cols[1] * C].rearrange("g (k c) -> g k c", c=C),
                    in0=PWALL[:, ea:eb].rearrange("g (k c) -> g k c", c=C),
                    in1=ONEHOT[:, None, :].to_broadcast([128, cb - ca, C]),
                    op=OP.mult)
                nc.vector.tensor_reduce(
                    PSEL[:, ca:cb],
                    P3[:, ea - _lvl_cols[1] * C:eb - _lvl_cols[1] * C].rearrange("g (k c) -> g k c", c=C),
                    axis=AX.X, op=OP.add)
            nc.scalar.activation(T1[:, ea:eb], PWALL[:, ea:eb], AF.Ln,
                                 bias=1.0, scale=-1.0)
            nc.scalar.activation(T2[:, ea:eb], PWALL[:, ea:eb], AF.Square)
            nc.vector.tensor_tensor(out=T2[:, ea:eb], in0=T2[:, ea:eb],
                                    in1=T1[:, ea:eb], op=OP.mult)
            nc.vector.tensor_reduce(
                F0W[:, ca:cb], T2[:, ea:eb].rearrange("g (k c) -> g k c", c=C),
                axis=AX.X, op=OP.add)

        # ---- focal window terms from psel ----------------------------------
        LN1 = pc.tile([128, NW], f32)
        nc.scalar.activation(LN1[:], PSEL[:], AF.Ln, bias=1.0, scale=-1.0)
        LNP = pc.tile([128, NW], f32)
        nc.scalar.activation(LNP[:], PSEL[:], AF.Ln)
        SQ = pc.tile([128, NW], f32)
        nc.scalar.activation(SQ[:], PSEL[:], AF.Square)
        SQ1 = pc.tile([128, NW], f32)
        nc.scalar.activation(SQ1[:], PSEL[:], AF.Square, bias=1.0, scale=-1.0)
        nc.vector.tensor_tensor(out=SQ1[:], in0=SQ1[:], in1=LNP[:], op=OP.mult)
        nc.vector.tensor_tensor(out=SQ[:], in0=SQ[:], in1=LN1[:], op=OP.mult)
        CONTR = pc.tile([128, NW], f32)
        nc.vector.scalar_tensor_tensor(CONTR[:], SQ1[:], 1.0 / 3.0, SQ[:],
                                       OP.mult, OP.subtract)

        # regr channel extraction: one permuted-AP copy per level
        # out [128, ch, b, j] <- band view [128, b, j, ch]
        for l in range(NLEV):
            nh, (fh, fw), w = NH[l], FS[l], W[l]
            vin = RBAND[:, _bofs[l]:_bofs[l] + nh * fw * 4].rearrange(
                "g (b x c) -> g b x c", x=fw, c=4)[:, :, 0:w, :].rearrange(
                "g b x c -> g c b x")
            vout = P4T[:].rearrange("g (c k) -> g c k", c=4)[
                :, :, _lvl_cols[l]:_lvl_cols[l + 1]].rearrange(
                "g c (b x) -> g c b x", x=w)
            nc.vector.tensor_copy(vout, vin)

        # ---- IoU on windows (batched over the 4 sides) ----------------------
        # scaled coords: sx' = 0.25*(xs+j+0.5) ; box edges scaled by 1/(4s)
        SXY2 = pc.tile([128, 2 * NW], f32)
        nc.vector.tensor_scalar(SXY2[:], XJROWY[:], 0.25, 0.125, OP.mult, OP.add)
        EDGEQ = pc.tile([128, 4 * NW], f32)
        nc.vector.tensor_tensor(
            out=EDGEQ[:].rearrange("g (q k) -> g q k", k=NW),
            in0=GT[:, 0:4, None].to_broadcast([128, 4, NW]),
            in1=INV4S296.rearrange("g (q k) -> g q k", k=NW),
            op=OP.mult)
        TLRB = pc.tile([128, 4 * NW], f32)
        nc.vector.tensor_tensor(out=TLRB[:, 0:2 * NW], in0=SXY2[:],
                                in1=EDGEQ[:, 0:2 * NW], op=OP.subtract)
        nc.vector.tensor_tensor(out=TLRB[:, 2 * NW:4 * NW], in0=EDGEQ[:, 2 * NW:4 * NW],
                                in1=SXY2[:], op=OP.subtract)
        nc.vector.tensor_scalar(TLRB[:], TLRB[:], 0.0, None, OP.max)

        TSUM = pc.tile([128, 2 * NW], f32)
        nc.vector.tensor_tensor(out=TSUM[:], in0=TLRB[:, 0:2 * NW],
                                in1=TLRB[:, 2 * NW:4 * NW], op=OP.add)
        TAREA = pc.tile([128, NW], f32)
        nc.vector.tensor_tensor(out=TAREA[:], in0=TSUM[:, 0:NW],
                                in1=TSUM[:, NW:2 * NW], op=OP.mult)
        PS = pc.tile([128, 2 * NW], f32)
        nc.vector.tensor_tensor(out=PS[:], in0=P4T[:, 0:2 * NW],
                                in1=P4T[:, 2 * NW:4 * NW], op=OP.add)
        PAREA = pc.tile([128, NW], f32)
        nc.vector.tensor_tensor(out=PAREA[:], in0=PS[:, 0:NW],
                                in1=PS[:, NW:2 * NW], op=OP.mult)
        MIN4 = pc.tile([128, 4 * NW], f32)
        nc.vector.tensor_tensor(out=MIN4[:], in0=P4T[:], in1=TLRB[:], op=OP.min)
        WIHI = pc.tile([128, 2 * NW], f32)
        nc.vector.tensor_tensor(out=WIHI[:], in0=MIN4[:, 0:2 * NW],
                                in1=MIN4[:, 2 * NW:4 * NW], op=OP.add)
        AI = pc.tile([128, NW], f32)
        nc.vector.tensor_tensor(out=AI[:], in0=WIHI[:, 0:NW],
                                in1=WIHI[:, NW:2 * NW], op=OP.mult)
        AU = pc.tile([128, NW], f32)
        nc.vector.tensor_tensor(out=AU[:], in0=TAREA[:], in1=PAREA[:], op=OP.add)
        nc.vector.tensor_tensor(out=AU[:], in0=AU[:], in1=AI[:], op=OP.subtract)
        LNAI = pc.tile([128, NW], f32)
        nc.scalar.activation(LNAI[:], AI[:], AF.Ln, bias=CONSTEPS)
        LNAU = pc.tile([128, NW], f32)
        nc.scalar.activation(LNAU[:], AU[:], AF.Ln, bias=CONSTEPS)
        LNR = pc.tile([128, NW], f32)
        nc.vector.tensor_tensor(out=LNR[:], in0=LNAI[:], in1=LNAU[:], op=OP.subtract)

        # ---- per-cell total and per-level masked sums ----------------------
        # SL_cell = mask * (0.75*(F0W + CONTR) + LNR)
        FCM = pc.tile([128, NW], f32)
        nc.vector.tensor_tensor(out=FCM[:], in0=F0W[:], in1=CONTR[:], op=OP.add)
        nc.vector.tensor_tensor(out=FCM[:], in0=FCM[:], in1=M74[:], op=OP.mult)
        LNM = pc.tile([128, NW], f32)
        nc.vector.tensor_tensor(out=LNM[:], in0=LNR[:], in1=M74[:], op=OP.mult)
        TOT = pc.tile([128, NW], f32)
        nc.vector.scalar_tensor_tensor(TOT[:], FCM[:], 0.75, LNM[:],
                                       OP.mult, OP.add)
        SL5 = pc.tile([128, NLEV], f32)
        nc.vector.tensor_reduce(SL5[:, 0:1], TOT[:, _lvl_cols[0]:_lvl_cols[1]],
                                axis=AX.X, op=OP.add)
        for l in range(1, NLEV):
            a, b = _lvl_cols[l], _lvl_cols[l + 1]
            nc.vector.tensor_reduce(SL5[:, l:l + 1], TOT[:, a:b],
                                    axis=AX.X, op=OP.add)

        # ---- combine halves, finalize loss, argmin -------------------------
        LVA = pc.tile([128, NLEV], f32)
        nc.vector.tensor_tensor(out=LVA[:], in0=SL5[:], in1=RECDN[:], op=OP.mult)
        EMXB = pc.tile([64, NLEV], f32)
        nc.vector.tensor_scalar(EMXB[:], EMX[0:64, :], BIG, None, OP.mult)
        SLH = pc.tile([64, NLEV], f32)
        nc.sync.dma_start(SLH[:], LVA[64:128, :])
        LOSSH0 = pc.tile([64, NLEV], f32)
        nc.vector.tensor_tensor(out=LOSSH0[:], in0=EMXB[:], in1=LVA[0:64, :],
                                op=OP.subtract)
        LOSS = pc.tile([64, NLEV], f32)
        nc.vector.tensor_tensor(out=LOSS[:], in0=LOSSH0[:], in1=SLH[:], op=OP.subtract)

        # argmin(LOSS) with first-match tie-break:
        # PEN = (l + BIG2) - BIG2*eq ; min PEN = argmin
        MBEST = pc.tile([64, 1], f32)
        nc.vector.tensor_reduce(MBEST[:], LOSS[:], axis=AX.X, op=OP.min)
        EQ5 = pc.tile([64, NLEV], f32)
        nc.vector.tensor_tensor(out=EQ5[:], in0=LOSS[:],
                                in1=MBEST[:, 0:1].to_broadcast([64, NLEV]),
                                op=OP.is_equal)
        PEN5 = pc.tile([64, NLEV], f32)
        nc.vector.scalar_tensor_tensor(PEN5[:], EQ5[:], -BIG2, CLB5[0:64, :],
                                       OP.mult, OP.add)
        IDX = pc.tile([64, 1], f32)
        nc.vector.tensor_reduce(IDX[:], PEN5[:], axis=AX.X, op=OP.min)
        nc.vector.copy_predicated(IDX[:], NV[0:64, :], CONSTM1[0:64, :])
        IDXI = pc.tile([64, 1], i32)
        nc.vector.tensor_copy(IDXI[:], IDX[:])
        nc.sync.dma_start(out_lvl.ap()[:, None], IDXI[:])

    nc.compile()
    return nc


_NC_CACHE = None


def _get_nc():
    global _NC_CACHE
    if _NC_CACHE is None:
        _NC_CACHE = build_nc(num_devices=8)
    return _NC_CACHE


def kernel(cls_pred, regr_pred, feature_shapes, gt_boxes):
    from concourse.bass_utils import run_bass_kernel_spmd

    B = cls_pred.shape[0]
    assert B == 8 and cls_pred.shape[1] == NLOC and cls_pred.shape[2] == C
    nc = _get_nc()
    in_maps = [
        {
            "cls_b": np.ascontiguousarray(cls_pred[b], dtype=np.float32),
            "regr_b": np.ascontiguousarray(regr_pred[b], dtype=np.float32),
            "gt_b": np.ascontiguousarray(gt_boxes[b], dtype=np.float32),
        }
        for b in range(B)
    ]
    res = run_bass_kernel_spmd(nc, in_maps, list(range(B)))
    out = np.stack([np.asarray(res.results[b]["out_lvl"]).reshape(G) for b in range(B)])
    return out.reshape(-1).astype(np.int32)



# revision 4
# speedup vs baseline: 1.0438x; 1.0438x over previous
"""Trainium2 Bass kernel for nn_MetaSelectTarget (FPN level assignment).

Strategy (v2):
  - Data-parallel over batch: B=8 images -> 8 NeuronCores, one image each.
  - 128 partitions: box g lives on partitions g and g+64, each half handling
    half of the window rows.  Unified column layout packs all levels' window
    cells into 70 slots.
  - Host-side layout prep (pure data movement): cls and regr are concatenated
    into one [NLOC, 84] array so the 12 row-block gathers fetch class AND
    regr data together; cls is also transposed to [80, NLOC] so the level-0
    per-gt-class probabilities come from ONE span gather (start =
    label*NLOC + y1*fw + x1, static in-span offsets b*fw+j).
  - psel for levels 1-4 via one-hot multiply + class reduce.
  - Focal f0 term: ACT does Ln(1-p) and Square(p); multiply is split
    DVE/Pool; per-slot class reduce on DVE.
  - Tail: halves combined with a DVE stream_shuffle (partition crossbar), no
    DMA; argmin + padding masking on-device.
"""

import numpy as np

import concourse.bass as bass
import concourse.bacc as bacc
import concourse.tile as tile
from concourse import mybir
from contextlib import ExitStack

f32 = mybir.dt.float32
i32 = mybir.dt.int32
u16 = mybir.dt.uint16
AF = mybir.ActivationFunctionType
OP = mybir.AluOpType
AX = mybir.AxisListType

# Problem constants (hardcoded per spec)
G = 64
C = 80
CC = 84                   # combined row: 80 cls + 4 regr
FS = [(128, 128), (64, 64), (32, 32), (16, 16), (8, 8)]
STRIDES = [8.0, 16.0, 32.0, 64.0, 128.0]
ROWOFS = [0, 16384, 20480, 21504, 21760]
NLOC = 21824
W = [9, 5, 3, 2, 2]       # max window span per level
NH = [5, 3, 2, 1, 1]      # window row-blocks handled per partition half
EPS = 1e-7
BIG = 1e7
BIG2 = 16.0               # argmin tie-break penalty (> NLEV)
NLEV = 5

# unified column layout: one column per (level, row-block, slot)
_cols = []      # (level, block, slot)
for _l in range(NLEV):
    for _b in range(NH[_l]):
        for _j in range(W[_l]):
            _cols.append((_l, _b, _j))
NW = len(_cols)           # 70
assert NW == 70
_lvl_cols = [sum(NH[l] * W[l] for l in range(lv)) for lv in range(NLEV + 1)]
# (level, block) -> (column of slot 0, index into the 12-entry block tables)
_J0 = [(l, b, k) for k, (l, b, j) in enumerate(_cols) if j == 0]
assert len(_J0) == 12
L14A, L14B = _lvl_cols[1], _lvl_cols[5]   # 45, 70
NL14 = L14B - L14A                        # 25


def _wrap_idx(mapv):
    """Per-16-partition wrapped uint16 index tensor for gpsimd.indirect_copy:
    unwrapped[i] lives at (i % 16, i // 16)."""
    n = len(mapv)
    s = (n + 15) // 16
    a = np.zeros((128, s), np.uint16)
    for i, m in enumerate(mapv):
        a[np.arange(8) * 16 + i % 16, i // 16] = m
    return a


def build_nc(num_devices=8):
    nc = bacc.Bacc("TRN2", target_bir_lowering=False, num_devices=num_devices)

    comb_b = nc.dram_tensor("comb_b", [NLOC, CC], f32, kind="ExternalInput")
    clsT_b = nc.dram_tensor("clsT_b", [C, NLOC], f32, kind="ExternalInput")
    gt_b = nc.dram_tensor("gt_b", [G, 5], f32, kind="ExternalOutput"
                          if False else "ExternalInput")
    out_lvl = nc.dram_tensor("out_lvl", [G], i32, kind="ExternalOutput")

    comb_flat = comb_b.ap().rearrange("n c -> (n c)")[None, :]
    clsT_flat = clsT_b.ap().rearrange("n c -> (n c)")[None, :]

    # ---- inline constants --------------------------------------------------
    # V-chain tables [128, NLEV*4]; cols j: 0=x1, 1=y1, 2=x2, 3=y2
    recip = np.zeros((128, NLEV, 4), np.float32)
    maskF = np.zeros((128, NLEV, 4), np.float32)
    maskC = np.zeros((128, NLEV, 4), np.float32)
    clo = np.zeros((128, NLEV, 4), np.float32)
    chi = np.zeros((128, NLEV, 4), np.float32)
    shi = np.zeros((128, NLEV, 4), np.float32)
    for l in range(NLEV):
        fh, fw = FS[l]
        w = W[l]
        recip[:, l, :] = 1.0 / STRIDES[l]
        maskF[:, l, 0] = maskF[:, l, 1] = 1.0
        maskC[:, l, 2] = maskC[:, l, 3] = 1.0
        clo[:, l, :] = [0.0, 0.0, 1.0, 1.0]
        chi[:, l, :] = [fw - 1, fh - 1, fw, fh]
        shi[:, l, :] = [fw - w, fh - w, 1e9, 1e9]

    # per-column tables
    r74 = np.zeros((128, NW), np.float32)     # window row index
    j74 = np.zeros((128, NW), np.float32)     # slot (x offset in window)
    inv4s296 = np.zeros((128, 4 * NW), np.float32)
    for k, (l, b, j) in enumerate(_cols):
        r74[0:64, k] = b
        r74[64:128, k] = NH[l] + b
        j74[:, k] = j
        for ch in range(4):
            inv4s296[:, ch * NW + k] = 1.0 / (4.0 * STRIDES[l])

    # row-block tables [128, 12]
    r12 = np.zeros((128, 12), np.float32)
    fw12 = np.zeros((128, 12), np.float32)
    fhm112 = np.zeros((128, 12), np.float32)
    rofs12 = np.zeros((128, 12), np.float32)
    for s, (l, b, k) in enumerate(_J0):
        r12[0:64, s] = b
        r12[64:128, s] = NH[l] + b
        fw12[:, s] = FS[l][1]
        fhm112[:, s] = FS[l][0] - 1
        rofs12[:, s] = ROWOFS[l]

    cconst = np.tile(np.arange(C, dtype=np.float32), (128, 1))
    clb5 = np.tile(np.arange(NLEV, dtype=np.float32) + BIG2, (128, 1))
    constm1 = np.full((128, 1), -1.0, np.float32)
    consteps = np.full((128, 1), EPS, np.float32)
    half640 = np.zeros((128, 1), np.float32)
    half640[64:128, 0] = float(NH[0] * FS[0][1])   # upper half: +5*128 cells

    consts = np.concatenate(
        [recip.reshape(128, -1), maskF.reshape(128, -1), maskC.reshape(128, -1),
         clo.reshape(128, -1), chi.reshape(128, -1), shi.reshape(128, -1),
         r74, j74, inv4s296,
         r12, fw12, fhm112, rofs12,
         cconst, clb5, constm1, consteps, half640], axis=1)
    t_consts = nc.inline_tensor(consts, "c_all")
    NCONST = consts.shape[1]

    # XSYS: out blocks [XS74, YS74, X1V74, Y1V74, X2V74, Y2V74, XS12, YS12]
    # from SVVR [128, 40] (SV at 0:20, VR at 20:40; (l j) layout)
    xsys_map = []
    for q in range(6):
        for k, (l, b, j) in enumerate(_cols):
            xsys_map.append([4 * l + 0, 4 * l + 1, 20 + 4 * l + 0, 20 + 4 * l + 1,
                             20 + 4 * l + 2, 20 + 4 * l + 3][q])
    for s, (l, b, k) in enumerate(_J0):
        xsys_map.append(4 * l + 0)
    for s, (l, b, k) in enumerate(_J0):
        xsys_map.append(4 * l + 1)
    t_xsys_idx_s = nc.inline_tensor(_wrap_idx(xsys_map[6 * NW:]), "xsys_idx_s")
    t_xsys_idx_b = nc.inline_tensor(_wrap_idx(xsys_map[:6 * NW]), "xsys_idx_b")
    NXS = len(xsys_map)  # 444

    NPW = NW * CC             # 5880 combined window data
    NF0 = NW * C              # 5600

    with tile.TileContext(nc) as tc, ExitStack() as ctx:
        pc = ctx.enter_context(tc.tile_pool(name="pc", bufs=1))

        GT = pc.tile([128, 5], f32)
        nc.sync.dma_start(GT[0:64, :], gt_b[:])
        nc.sync.dma_start(GT[64:128, :], gt_b[:])
        CST = pc.tile([128, NCONST], f32)
        nc.scalar.dma_start(CST[:], t_consts[:])
        XSYS_IDX_S = pc.tile([128, 2], u16)
        XSYS_IDX_B = pc.tile([128, (6 * NW + 15) // 16], u16)
        nc.sync.dma_start(XSYS_IDX_S[:], t_xsys_idx_s[:])
        nc.sync.dma_start(XSYS_IDX_B[:], t_xsys_idx_b[:])
        # dummy ops: force the Ln/Square/Sqrt ACT table load at t~0
        WARM = pc.tile([1, 2], f32)
        nc.vector.memset(WARM[:], 0.5)
        nc.scalar.activation(WARM[:, 0:1], WARM[:, 0:1], AF.Ln)
        nc.scalar.activation(WARM[:, 1:2], WARM[:, 1:2], AF.Square)

        off = 0
        def _cview(n):
            nonlocal off
            v = CST[:, off:off + n]
            off += n
            return v
        RECIP = _cview(NLEV * 4)
        MASKF = _cview(NLEV * 4)
        MASKC = _cview(NLEV * 4)
        CLO = _cview(NLEV * 4)
        CHI = _cview(NLEV * 4)
        SHI = _cview(NLEV * 4)
        R74 = _cview(NW)
        J74 = _cview(NW)
        INV4S296 = _cview(4 * NW)
        R12 = _cview(12)
        FW12 = _cview(12)
        FHM112 = _cview(12)
        ROFS12 = _cview(12)
        CCONST = _cview(C)
        CLB5 = _cview(NLEV)
        CONSTM1 = _cview(1)
        CONSTEPS = _cview(1)
        HALF640 = _cview(1)

        LBL = pc.tile([128, 1], f32)
        nc.vector.tensor_scalar(LBL[:], GT[:, 4:5], 0.0, float(C - 1), OP.max, OP.min)

        # ---- box math (both halves compute identical values) ---------------
        # shrunk box edges: q = 0.6*edge + 0.4*opposite
        GTSW = pc.tile([128, 4], f32)
        nc.vector.tensor_copy(GTSW[:, 0:2], GT[:, 2:4])
        nc.vector.tensor_copy(GTSW[:, 2:4], GT[:, 0:2])
        Q = pc.tile([128, 4], f32)
        nc.vector.tensor_scalar(Q[:], GTSW[:], 0.4, None, OP.mult)
        nc.vector.scalar_tensor_tensor(Q[:], GT[:, 0:4], 0.6, Q[:], OP.mult, OP.add)

        # V[p, l, j] = Q[p, j] / stride_l ; robust floor/ceil ; clip
        SVVR = pc.tile([128, 40], f32)
        SV = SVVR[:, 0:20]
        VR = SVVR[:, 20:40]
        V = pc.tile([128, NLEV * 4], f32)
        nc.vector.tensor_tensor(
            out=V[:].rearrange("g (l j) -> g l j", j=4),
            in0=Q[:, None, :].to_broadcast([128, NLEV, 4]),
            in1=RECIP.rearrange("g (l j) -> g l j", j=4),
            op=OP.mult,
        )
        VI = pc.tile([128, NLEV * 4], i32)
        nc.vector.tensor_copy(VI[:], V[:])
        VF = pc.tile([128, NLEV * 4], f32)
        nc.vector.tensor_copy(VF[:], VI[:])
        GG = pc.tile([128, NLEV * 4], f32)
        nc.vector.tensor_tensor(out=GG[:], in0=VF[:], in1=V[:], op=OP.is_gt)
        LL = pc.tile([128, NLEV * 4], f32)
        nc.vector.tensor_tensor(out=LL[:], in0=VF[:], in1=V[:], op=OP.is_lt)
        nc.vector.tensor_tensor(out=GG[:], in0=GG[:], in1=MASKF, op=OP.mult)
        nc.vector.tensor_tensor(out=LL[:], in0=LL[:], in1=MASKC, op=OP.mult)
        nc.vector.tensor_tensor(out=VR, in0=VF[:], in1=GG[:], op=OP.subtract)
        nc.vector.tensor_tensor(out=VR, in0=VR, in1=LL[:], op=OP.add)
        nc.vector.tensor_tensor(out=VR, in0=VR, in1=CLO, op=OP.max)
        nc.vector.tensor_tensor(out=VR, in0=VR, in1=CHI, op=OP.min)
        nc.vector.tensor_tensor(out=SV, in0=VR, in1=SHI, op=OP.min)

        # ---- row-block start cells (critical path to the gathers) ----------
        # cell = min(ys + r, fh-1)*fw + xs + rowofs
        XSYS = pc.tile([128, NXS], f32)
        nc.gpsimd.indirect_copy(XSYS[:, 6 * NW:NXS], SVVR[:], XSYS_IDX_S[:], True)
        XS12 = XSYS[:, 6 * NW:6 * NW + 12]
        YS12 = XSYS[:, 6 * NW + 12:6 * NW + 24]
        CELL12 = pc.tile([128, 12], f32)
        nc.vector.tensor_tensor(out=CELL12[:], in0=YS12, in1=R12, op=OP.add)
        nc.vector.tensor_tensor(out=CELL12[:], in0=CELL12[:], in1=FHM112, op=OP.min)
        nc.vector.tensor_tensor(out=CELL12[:], in0=CELL12[:], in1=FW12, op=OP.mult)
        nc.vector.tensor_tensor(out=CELL12[:], in0=CELL12[:], in1=XS12, op=OP.add)
        nc.vector.tensor_tensor(out=CELL12[:], in0=CELL12[:], in1=ROFS12, op=OP.add)
        RICF = pc.tile([128, 12], f32)
        nc.vector.tensor_scalar(RICF[:], CELL12[:], float(CC), None, OP.mult)
        RIC = pc.tile([128, 12], i32)
        nc.vector.tensor_copy(RIC[:], RICF[:])

        # psel level-0 span gather start:
        # label*NLOC + y1*fw0 + x1 (+ 5*fw0 for the upper half)
        PSIDXF = pc.tile([128, 1], f32)
        nc.vector.tensor_scalar(PSIDXF[:], LBL[:], float(NLOC), None, OP.mult)
        nc.vector.scalar_tensor_tensor(PSIDXF[:], VR[:, 1:2], float(FS[0][1]),
                                       PSIDXF[:], OP.mult, OP.add)
        nc.vector.tensor_tensor(out=PSIDXF[:], in0=PSIDXF[:], in1=VR[:, 0:1],
                                op=OP.add)
        nc.vector.tensor_tensor(out=PSIDXF[:], in0=PSIDXF[:], in1=HALF640,
                                op=OP.add)
        PSIDX = pc.tile([128, 1], i32)
        nc.vector.tensor_copy(PSIDX[:], PSIDXF[:])

        # ---- gathers (SWDGE): one contiguous block per partition -----------
        PWALL = pc.tile([128, NPW], f32)
        PWv = PWALL[:].rearrange("g (k c) -> g k c", c=CC)
        PSL0 = pc.tile([128, 5 * FS[0][1]], f32)
        for s, (l, b, k) in enumerate(_J0):
            ca = _lvl_cols[l] + b * W[l]
            nc.gpsimd.indirect_dma_start(
                out=PWALL[:, ca * CC:(ca + W[l]) * CC], out_offset=None,
                in_=comb_flat,
                in_offset=bass.IndirectOffsetOnAxis(ap=RIC[:, s:s + 1], axis=1))
        nc.gpsimd.indirect_dma_start(
            out=PSL0[:, 0:576], out_offset=None, in_=clsT_flat,
            in_offset=bass.IndirectOffsetOnAxis(ap=PSIDX[:, 0:1], axis=1))
        nc.gpsimd.indirect_copy(XSYS[:, 0:6 * NW], SVVR[:], XSYS_IDX_B[:], True)
        XS74 = XSYS[:, 0 * NW:1 * NW]
        YS74 = XSYS[:, 1 * NW:2 * NW]
        X1Y1V = XSYS[:, 2 * NW:4 * NW]
        X2Y2V = XSYS[:, 4 * NW:6 * NW]

        # ---- window mask [128, NW] -----------------------------------------
        XJROWY = pc.tile([128, 2 * NW], f32)
        nc.vector.tensor_tensor(out=XJROWY[:, 0:NW], in0=XS74, in1=J74, op=OP.add)
        nc.vector.tensor_tensor(out=XJROWY[:, NW:2 * NW], in0=YS74, in1=R74,
                                op=OP.add)
        MGE = pc.tile([128, 2 * NW], f32)
        nc.vector.tensor_tensor(out=MGE[:], in0=XJROWY[:], in1=X1Y1V, op=OP.is_ge)
        MLT = pc.tile([128, 2 * NW], f32)
        nc.vector.tensor_tensor(out=MLT[:], in0=XJROWY[:], in1=X2Y2V, op=OP.is_lt)
        nc.vector.tensor_tensor(out=MGE[:], in0=MGE[:], in1=MLT[:], op=OP.mult)
        M74 = pc.tile([128, NW], f32)
        nc.vector.tensor_tensor(out=M74[:], in0=MGE[:, 0:NW], in1=MGE[:, NW:2 * NW],
                                op=OP.mult)

        # empty / denom per level [128, NLEV] (rows 0:64 used at the end)
        VR3 = VR.rearrange("g (l j) -> g l j", j=4)
        x1v, y1v, x2v, y2v = VR3[:, :, 0], VR3[:, :, 1], VR3[:, :, 2], VR3[:, :, 3]
        EX = pc.tile([128, NLEV], f32)
        nc.vector.tensor_tensor(out=EX[:], in0=x1v, in1=x2v, op=OP.is_equal)
        EY = pc.tile([128, NLEV], f32)
        nc.vector.tensor_tensor(out=EY[:], in0=y1v, in1=y2v, op=OP.is_equal)
        EMX = pc.tile([128, NLEV], f32)
        nc.vector.tensor_tensor(out=EMX[:], in0=EX[:], in1=EY[:], op=OP.max)
        DX = pc.tile([128, NLEV], f32)
        nc.vector.tensor_tensor(out=DX[:], in0=x2v, in1=x1v, op=OP.subtract)
        DY = pc.tile([128, NLEV], f32)
        nc.vector.tensor_tensor(out=DY[:], in0=y2v, in1=y1v, op=OP.subtract)
        DN = pc.tile([128, NLEV], f32)
        nc.vector.tensor_tensor(out=DN[:], in0=DX[:], in1=DY[:], op=OP.mult)
        nc.vector.tensor_scalar(DN[:], DN[:], 1.0, None, OP.max)
        RECDN = pc.tile([128, NLEV], f32)
        nc.vector.reciprocal(RECDN[:], DN[:])

        ONEHOT = pc.tile([128, C], f32)
        nc.vector.tensor_tensor(out=ONEHOT[:], in0=CCONST,
                                in1=LBL[:, 0:1].to_broadcast([128, C]),
                                op=OP.is_equal)
        SABS = pc.tile([128, 1], f32)
        nc.vector.tensor_reduce(SABS[:], GT[:, 0:4], axis=AX.X, op=OP.add,
                                apply_absolute_value=True)
        NV = pc.tile([128, 1], i32)
        nc.vector.tensor_scalar(NV[:], SABS[:], 0.0, None, OP.is_le)

        # ---- f0 term: per-slot sum over classes of p^2 ln(1-p) --------------
        # regions follow the gather order so compute pipelines behind DMA
        T1 = pc.tile([128, NF0], f32)
        T2 = pc.tile([128, NF0], f32)
        F0W = pc.tile([128, NW], f32)
        # region list: (slot range) per level-0 block pair, then levels 1-4
        regions = [(b * W[0], (b + 1) * W[0]) for b in range(NH[0])]
        for l in range(1, NLEV):
            regions.append((_lvl_cols[l], _lvl_cols[l + 1]))
        for ri, (ca, cb) in enumerate(regions):
            nk = cb - ca
            pv = PWv[:, ca:cb, 0:C]
            nc.scalar.activation(
                T1[:, ca * C:cb * C].rearrange("g (k c) -> g k c", c=C),
                pv, AF.Ln, bias=1.0, scale=-1.0)
            nc.scalar.activation(
                T2[:, ca * C:cb * C].rearrange("g (k c) -> g k c", c=C),
                pv, AF.Square)
            nc.vector.tensor_tensor(
                out=T2[:, ca * C:cb * C], in0=T2[:, ca * C:cb * C],
                in1=T1[:, ca * C:cb * C], op=OP.mult)
            nc.vector.tensor_reduce(
                F0W[:, ca:cb],
                T2[:, ca * C:cb * C].rearrange("g (k c) -> g k c", c=C),
                axis=AX.X, op=OP.add)

        # ---- psel: level 0 from the span gather, levels 1-4 one-hot ---------
        PSEL = pc.tile([128, NW], f32)
        PSL0v = PSL0[:].rearrange("g (b x) -> g b x", x=FS[0][1])
        nc.scalar.copy(PSEL[:, 0:L14A].rearrange("g (b j) -> g b j", j=W[0]),
                       PSL0v[:, :, 0:W[0]])
        P3 = pc.tile([128, NL14 * C], f32)
        nc.vector.tensor_tensor(
            out=P3[:].rearrange("g (k c) -> g k c", c=C),
            in0=PWv[:, L14A:L14B, 0:C],
            in1=ONEHOT[:, None, :].to_broadcast([128, NL14, C]),
            op=OP.mult)
        nc.vector.tensor_reduce(
            PSEL[:, L14A:L14B], P3[:].rearrange("g (k c) -> g k c", c=C),
            axis=AX.X, op=OP.add)

        # ---- focal window terms from psel ----------------------------------
        LN1 = pc.tile([128, NW], f32)
        nc.scalar.activation(LN1[:], PSEL[:], AF.Ln, bias=1.0, scale=-1.0)
        LNP = pc.tile([128, NW], f32)
        nc.scalar.activation(LNP[:], PSEL[:], AF.Ln)
        SQ = pc.tile([128, NW], f32)
        nc.scalar.activation(SQ[:], PSEL[:], AF.Square)
        SQ1 = pc.tile([128, NW], f32)
        nc.scalar.activation(SQ1[:], PSEL[:], AF.Square, bias=1.0, scale=-1.0)
        nc.vector.tensor_tensor(out=SQ1[:], in0=SQ1[:], in1=LNP[:], op=OP.mult)
        nc.vector.tensor_tensor(out=SQ[:], in0=SQ[:], in1=LN1[:], op=OP.mult)
        CONTR = pc.tile([128, NW], f32)
        nc.vector.scalar_tensor_tensor(CONTR[:], SQ1[:], 1.0 / 3.0, SQ[:],
                                       OP.mult, OP.subtract)

        # regr channel extraction: one strided copy per level
        P4T = pc.tile([128, 4 * NW], f32)
        P4Tv = P4T[:].rearrange("g (c k) -> g c k", c=4)
        for l in range(NLEV):
            a, b = _lvl_cols[l], _lvl_cols[l + 1]
            nc.vector.tensor_copy(
                P4Tv[:, :, a:b],
                PWv[:, a:b, C:CC].rearrange("g k c -> g c k"))

        # ---- IoU on windows (batched over the 4 sides) ----------------------
        SXY2 = pc.tile([128, 2 * NW], f32)
        nc.vector.tensor_scalar(SXY2[:], XJROWY[:], 0.25, 0.125, OP.mult, OP.add)
        EDGEQ = pc.tile([128, 4 * NW], f32)
        nc.vector.tensor_tensor(
            out=EDGEQ[:].rearrange("g (q k) -> g q k", k=NW),
            in0=GT[:, 0:4, None].to_broadcast([128, 4, NW]),
            in1=INV4S296.rearrange("g (q k) -> g q k", k=NW),
            op=OP.mult)
        TLRB = pc.tile([128, 4 * NW], f32)
        nc.vector.tensor_tensor(out=TLRB[:, 0:2 * NW], in0=SXY2[:],
                                in1=EDGEQ[:, 0:2 * NW], op=OP.subtract)
        nc.vector.tensor_tensor(out=TLRB[:, 2 * NW:4 * NW],
                                in0=EDGEQ[:, 2 * NW:4 * NW],
                                in1=SXY2[:], op=OP.subtract)
        nc.vector.tensor_scalar(TLRB[:], TLRB[:], 0.0, None, OP.max)

        TSUM = pc.tile([128, 2 * NW], f32)
        nc.vector.tensor_tensor(out=TSUM[:], in0=TLRB[:, 0:2 * NW],
                                in1=TLRB[:, 2 * NW:4 * NW], op=OP.add)
        TAREA = pc.tile([128, NW], f32)
        nc.vector.tensor_tensor(out=TAREA[:], in0=TSUM[:, 0:NW],
                                in1=TSUM[:, NW:2 * NW], op=OP.mult)
        PS = pc.tile([128, 2 * NW], f32)
        nc.vector.tensor_tensor(out=PS[:], in0=P4T[:, 0:2 * NW],
                                in1=P4T[:, 2 * NW:4 * NW], op=OP.add)
        PAREA = pc.tile([128, NW], f32)
        nc.vector.tensor_tensor(out=PAREA[:], in0=PS[:, 0:NW],
                                in1=PS[:, NW:2 * NW], op=OP.mult)
        MIN4 = pc.tile([128, 4 * NW], f32)
        nc.vector.tensor_tensor(out=MIN4[:], in0=P4T[:], in1=TLRB[:], op=OP.min)
        WIHI = pc.tile([128, 2 * NW], f32)
        nc.vector.tensor_tensor(out=WIHI[:], in0=MIN4[:, 0:2 * NW],
                                in1=MIN4[:, 2 * NW:4 * NW], op=OP.add)
        AI = pc.tile([128, NW], f32)
        nc.vector.tensor_tensor(out=AI[:], in0=WIHI[:, 0:NW],
                                in1=WIHI[:, NW:2 * NW], op=OP.mult)
        AU = pc.tile([128, NW], f32)
        nc.vector.tensor_tensor(out=AU[:], in0=TAREA[:], in1=PAREA[:], op=OP.add)
        nc.vector.tensor_tensor(out=AU[:], in0=AU[:], in1=AI[:], op=OP.subtract)
        LNAI = pc.tile([128, NW], f32)
        nc.scalar.activation(LNAI[:], AI[:], AF.Ln, bias=CONSTEPS)
        LNAU = pc.tile([128, NW], f32)
        nc.scalar.activation(LNAU[:], AU[:], AF.Ln, bias=CONSTEPS)
        LNR = pc.tile([128, NW], f32)
        nc.vector.tensor_tensor(out=LNR[:], in0=LNAI[:], in1=LNAU[:],
                                op=OP.subtract)

        # ---- per-cell total and per-level masked sums ----------------------
        # SL_cell = mask * (0.75*(F0W + CONTR) + LNR)
        FCM = pc.tile([128, NW], f32)
        nc.vector.tensor_tensor(out=FCM[:], in0=F0W[:], in1=CONTR[:], op=OP.add)
        nc.vector.tensor_tensor(out=FCM[:], in0=FCM[:], in1=M74[:], op=OP.mult)
        LNM = pc.tile([128, NW], f32)
        nc.vector.tensor_tensor(out=LNM[:], in0=LNR[:], in1=M74[:], op=OP.mult)
        TOT = pc.tile([128, NW], f32)
        nc.vector.scalar_tensor_tensor(TOT[:], FCM[:], 0.75, LNM[:],
                                       OP.mult, OP.add)
        SL5 = pc.tile([128, NLEV], f32)
        for l in range(NLEV):
            a, b = _lvl_cols[l], _lvl_cols[l + 1]
            nc.vector.tensor_reduce(SL5[:, l:l + 1], TOT[:, a:b],
                                    axis=AX.X, op=OP.add)

        # ---- combine halves (stream shuffle), finalize loss, argmin --------
        LVA = pc.tile([128, NLEV], f32)
        nc.vector.tensor_tensor(out=LVA[:], in0=SL5[:], in1=RECDN[:], op=OP.mult)
        EMXB = pc.tile([64, NLEV], f32)
        nc.vector.tensor_scalar(EMXB[:], EMX[0:64, :], BIG, None, OP.mult)
        SLH = pc.tile([64, NLEV], f32)
        nc.vector.stream_shuffle(SLH[:], LVA[64:128, :], list(range(32)))
        LOSSH0 = pc.tile([64, NLEV], f32)
        nc.vector.tensor_tensor(out=LOSSH0[:], in0=EMXB[:], in1=LVA[0:64, :],
                                op=OP.subtract)
        LOSS = pc.tile([64, NLEV], f32)
        nc.vector.tensor_tensor(out=LOSS[:], in0=LOSSH0[:], in1=SLH[:],
                                op=OP.subtract)

        # argmin(LOSS) with first-match tie-break:
        # PEN = (l + BIG2) - BIG2*eq ; min PEN = argmin
        MBEST = pc.tile([64, 1], f32)
        nc.vector.tensor_reduce(MBEST[:], LOSS[:], axis=AX.X, op=OP.min)
        EQ5 = pc.tile([64, NLEV], f32)
        nc.vector.tensor_tensor(out=EQ5[:], in0=LOSS[:],
                                in1=MBEST[:, 0:1].to_broadcast([64, NLEV]),
                                op=OP.is_equal)
        PEN5 = pc.tile([64, NLEV], f32)
        nc.vector.scalar_tensor_tensor(PEN5[:], EQ5[:], -BIG2, CLB5[0:64, :],
                                       OP.mult, OP.add)
        IDX = pc.tile([64, 1], f32)
        nc.vector.tensor_reduce(IDX[:], PEN5[:], axis=AX.X, op=OP.min)
        nc.vector.copy_predicated(IDX[:], NV[0:64, :], CONSTM1[0:64, :])
        IDXI = pc.tile([64, 1], i32)
        nc.vector.tensor_copy(IDXI[:], IDX[:])
        nc.sync.dma_start(out_lvl.ap()[:, None], IDXI[:])

    nc.compile()
    return nc


_NC_CACHE = None


def _get_nc():
    global _NC_CACHE
    if _NC_CACHE is None:
        _NC_CACHE = build_nc(num_devices=8)
    return _NC_CACHE


def kernel(cls_pred, regr_pred, feature_shapes, gt_boxes):
    from concourse.bass_utils import run_bass_kernel_spmd

    B = cls_pred.shape[0]
    assert B == 8 and cls_pred.shape[1] == NLOC and cls_pred.shape[2] == C
    nc = _get_nc()
    cls_pred = np.asarray(cls_pred, dtype=np.float32)
    regr_pred = np.asarray(regr_pred, dtype=np.float32)
    gt_boxes = np.asarray(gt_boxes, dtype=np.float32)
    in_maps = [
        {
            "comb_b": np.ascontiguousarray(
                np.concatenate([cls_pred[b], regr_pred[b]], axis=-1)),
            "clsT_b": np.ascontiguousarray(cls_pred[b].T),
            "gt_b": np.ascontiguousarray(gt_boxes[b]),
        }
        for b in range(B)
    ]
    res = run_bass_kernel_spmd(nc, in_maps, list(range(B)))
    out = np.stack([np.asarray(res.results[b]["out_lvl"]).reshape(G)
                    for b in range(B)])
    return out.reshape(-1).astype(np.int32)
